# revision 8
# baseline (speedup 1.0000x reference)
"""CustomGaussianLayer Trainium2 kernel (bf16 rewrite).

Math: out[b,o] = sum_{i,g} exp(-a*(tanh(x[b,i])-c_g)^2) * coeff[o,i,g]*W[o,i]
 == E @ W2T  with  E[b, k=(i,g)] Gaussian basis,  W2T[k, o] folded weights,
 a = 24.5, centers c_g = linspace(-1,1,8).

Factored basis:  exp(-a(t-c)^2) = A * B_g * exp(-a c^2)
  A   = exp(-a t^2)                (ACT)
  B_g = exp(2 a c_g t)             even g: ACT exp; odd g: B_{g-1} * r (DVE)
  r   = exp(2 a dc t) = exp(14 t)  (ACT),  exp(-a c^2) folded into W2T (host)

Per core (data-parallel over batch, 1024 rows each): E in bf16, W2T in bf16,
PE accumulates fp32 into 8 psum banks [4 o-tiles x 2 b-chunks]; out stored
bf16 and upcast on host.  PE warm-up matmuls on memset scratch (independent
of DMA) defeat the HAM cold clock; xt DMA rides the GpSimd queue, w2 the
Sync queue.  First half (i-blocks 0-1) runs ib-major with a 512-col startup
sliver so real matmuls begin ~9.5us.
"""

import numpy as np
import ml_dtypes

import concourse.bacc as bacc
import concourse.bass as bass
import concourse.mybir as mybir
import concourse.tile as tile
from concourse.bass_utils import run_bass_kernel_spmd
from concourse.tile import add_dep_helper

G = 8
I_SZ = 512
O_SZ = 512
B = 8192
NCORES = 8
B_SH = B // NCORES          # 1024 batch rows per core
K = I_SZ * G                # 4096 contraction
N_IBLK = I_SZ // 128        # 4 partition blocks of i
FREE = N_IBLK * B_SH        # 4096 free layout (i_blk, b)
HALF = FREE // 2            # 2048 (i_blk 0-1 | 2-3)
N_OT = O_SZ // 128          # 4 output tiles
N_BC = B_SH // 512          # 2 batch chunks of 512 (psum free limit fp32)

ALPHA = 24.5
N_WARMUP = 9                # dummy matmuls to beat the HAM cold clock
CENTERS = np.linspace(-1.0, 1.0, G).astype(np.float64)
DC = float(CENTERS[1] - CENTERS[0])          # 2/7
R_SCALE = float(2.0 * ALPHA * DC)            # 14.0

F32 = mybir.dt.float32
BF16 = mybir.dt.bfloat16
AF = mybir.ActivationFunctionType
ALU = mybir.AluOpType

_NC_CACHE = {}


def build_nc():
    nc = bacc.Bacc("TRN2", target_bir_lowering=False)
    xt_d = nc.dram_tensor("xt", [I_SZ, B_SH], F32, kind="ExternalInput")
    w2t_d = nc.dram_tensor("w2t", [K, O_SZ], BF16, kind="ExternalInput")
    out_d = nc.dram_tensor("out_t", [O_SZ, B_SH], BF16, kind="ExternalOutput")

    with tile.TileContext(nc) as tc:
        with (
            tc.tile_pool(name="scr", bufs=1) as scr_pool,
            tc.tile_pool(name="w2", bufs=1) as w2_pool,
            tc.tile_pool(name="xt", bufs=1) as xt_pool,
            tc.tile_pool(name="tt", bufs=1) as tt_pool,
            tc.tile_pool(name="sq", bufs=2) as sq_pool,
            tc.tile_pool(name="rr", bufs=2) as rr_pool,
            tc.tile_pool(name="aa", bufs=2) as aa_pool,
            tc.tile_pool(name="sqh", bufs=1) as sqh_pool,
            tc.tile_pool(name="rrh", bufs=1) as rrh_pool,
            tc.tile_pool(name="aah", bufs=1) as aah_pool,
            tc.tile_pool(name="bb", bufs=2) as bb_pool,
            tc.tile_pool(name="bbh", bufs=2) as bbh_pool,
            tc.tile_pool(name="ee", bufs=8) as ee_pool,
            tc.tile_pool(name="ps", bufs=1, space="PSUM") as ps_pool,
            tc.tile_pool(name="ob", bufs=1) as ob_pool,
        ):
            # ---- warm-up scaffolding, all DMA-independent ----
            scr = scr_pool.tile([128, 640], BF16, tag="scr")
            nc.vector.memset(scr[:], 0.0)
            # ACT spline-table preload (exp/tanh share one table set)
            actwarm = scr_pool.tile([128, 2], F32, tag="actwarm")
            nc.vector.memset(actwarm[:, 0:1], 0.0)
            actwarm_i = nc.scalar.activation(
                actwarm[:, 1:2], actwarm[:, 0:1], AF.Exp)

            psum = [
                [
                    ps_pool.tile(
                        [128, 512], F32,
                        name=f"ps{ot}_{bc}", tag=f"ps{ot}_{bc}",
                    )
                    for bc in range(N_BC)
                ]
                for ot in range(N_OT)
            ]
            wu_is = []
            for w in range(N_WARMUP):
                wu_is.append(nc.tensor.matmul(
                    psum[0][0][:], scr[:, 0:128], scr[:, 128:640],
                    start=(w == 0), stop=(w == N_WARMUP - 1),
                ))

            # ---- input DMA: xt on GpSimd queue, w2 on Sync queue ----
            w2_all = w2_pool.tile([128, (K // 128) * O_SZ], BF16, tag="w2all")
            w2t_v = w2t_d[:, :].rearrange("(kt p) o -> p kt o", p=128)
            xt_sb = xt_pool.tile([128, FREE], F32, tag="xt")
            xt_v = xt_d[:, :].rearrange("(ib p) b -> p ib b", p=128)

            def w2_dma(kt_lo, kt_hi):
                return nc.sync.dma_start(
                    w2_all[:, kt_lo * O_SZ:kt_hi * O_SZ]
                    .rearrange("p (kt o) -> p kt o", o=O_SZ),
                    w2t_v[:, kt_lo:kt_hi, :],
                )

            xt_chain = [
                # (ib0, b 0:512) startup sliver, (ib0, b 512:1024), ib1, ib2-3
                nc.gpsimd.dma_start(xt_sb[:, 0:512], xt_v[:, 0, 0:512]),
                nc.gpsimd.dma_start(xt_sb[:, 512:1024], xt_v[:, 0, 512:1024]),
                nc.gpsimd.dma_start(
                    xt_sb[:, 1024:2048], xt_v[:, 1, :]),
                nc.gpsimd.dma_start(
                    xt_sb[:, 2048:4096]
                    .rearrange("p (ib b) -> p ib b", b=B_SH),
                    xt_v[:, 2:4, :]),
            ]
            w2_chain = [
                w2_dma(0, 2),      # h0-ib0 g0,g1
                w2_dma(2, 8),      # h0-ib0 g2-7
                w2_dma(8, 16),     # h0-ib1 g0-7
                w2_dma(16, 32),    # h1
            ]
            for ch in (xt_chain, w2_chain):
                for i in range(1, len(ch)):
                    add_dep_helper(ch[i].ins, ch[i - 1].ins, sync=False,
                                   reason="DMA lane consumer order")

            # ---- basis + matmuls ----
            # tt/xt col layout: (ib, b) ib0=0:1024 ib1=1024:2048 ib2.. ib3..
            mm_count = [0]
            TOTAL_MM = 2 * G * 2 * N_OT * N_BC

            def mms(e_ap, kt, tag_first_bank):
                """8 matmuls (ot x bc) consuming e_ap = [128, 1024] slice."""
                for ot in range(N_OT):
                    lhsT = w2_all[:, kt * O_SZ + ot * 128: kt * O_SZ + (ot + 1) * 128]
                    for bc in range(N_BC):
                        first = mm_count[0] < N_OT * N_BC
                        last = mm_count[0] >= TOTAL_MM - N_OT * N_BC
                        nc.tensor.matmul(
                            psum[ot][bc][:],
                            lhsT,
                            e_ap[:, bc * 512:(bc + 1) * 512],
                            start=first, stop=last,
                        )
                        mm_count[0] += 1

            act_chain = [actwarm_i]

            def act(out_ap, in_ap, fn, scale=1.0):
                i = nc.scalar.activation(out_ap, in_ap, fn, scale=float(scale))
                if act_chain:
                    add_dep_helper(i.ins, act_chain[-1].ins, sync=False,
                                   reason="ACT program order")
                act_chain.append(i)
                return i

            # --- h0 (ib0 + ib1), ib-major so we can start on ib0 slivers ---
            h0_parts = [(0, 512), (512, 1024)]           # ib0 slivers
            tt = tt_pool.tile([128, FREE], F32, tag="tt")
            sq0 = sq_pool.tile([128, 1024], F32, tag="sq")
            a0 = aa_pool.tile([128, 1024], F32, tag="aa")
            r0 = rr_pool.tile([128, 1024], F32, tag="rr")
            b_h0 = {}
            e_h0 = [ee_pool.tile([128, 2048], BF16, tag="ee", name=f"e_h0_{g}")
                    for g in range(G)]

            # ib0: startup slivers for g0, then full-width g1..g7
            for (lo, hi) in h0_parts:
                sl = slice(lo, hi)
                act(tt[:, sl], xt_sb[:, sl], AF.Tanh)
                c0 = float(CENTERS[0])
                if (lo, hi) == h0_parts[0]:
                    bt = bb_pool.tile([128, 1024], F32, tag="bb", name="b_h0_g0")
                else:
                    bt = b_h0[0]
                b_h0[0] = bt
                act(bt[:, sl], tt[:, sl], AF.Exp, scale=2.0 * ALPHA * c0)
                act(r0[:, sl], tt[:, sl], AF.Exp, scale=R_SCALE)
                nc.vector.tensor_tensor(
                    sq0[:, sl], tt[:, sl], tt[:, sl], op=ALU.mult)
                act(a0[:, sl], sq0[:, sl], AF.Exp, scale=-ALPHA)
                nc.vector.tensor_tensor(
                    e_h0[0][:, sl], a0[:, sl], b_h0[0][:, sl], op=ALU.mult)

            for g in range(1, G):
                c = float(CENTERS[g])
                if g % 2 == 1:
                    bt = bb_pool.tile([128, 1024], F32, tag="bb")
                    nc.vector.tensor_tensor(
                        bt[:], b_h0[g - 1][:], r0[:], op=ALU.mult)
                else:
                    bt = bb_pool.tile([128, 1024], F32, tag="bb")
                    act(bt[:], tt[:, 0:1024], AF.Exp, scale=2.0 * ALPHA * c)
                b_h0[g] = bt
                nc.vector.tensor_tensor(
                    e_h0[g][:, 0:1024], a0[:], bt[:], op=ALU.mult)

            # PE: ib0 sweep g0..g7 (kt = g)
            for g in range(G):
                mms(e_h0[g][:, 0:1024], g, g == 0)

            # ib1 basis (cols 1024:2048), then ib1 sweep (kt = 8+g)
            sl = slice(1024, 2048)
            sq1 = sq_pool.tile([128, 1024], F32, tag="sq")
            a1 = aa_pool.tile([128, 1024], F32, tag="aa")
            r1 = rr_pool.tile([128, 1024], F32, tag="rr")
            act(tt[:, sl], xt_sb[:, sl], AF.Tanh)
            b_prev = None
            for g in range(G):
                c = float(CENTERS[g])
                if g == 0:
                    bt = bb_pool.tile([128, 1024], F32, tag="bb")
                    act(bt[:], tt[:, sl], AF.Exp, scale=2.0 * ALPHA * c)
                    act(r1[:], tt[:, sl], AF.Exp, scale=R_SCALE)
                    nc.vector.tensor_tensor(
                        sq1[:], tt[:, sl], tt[:, sl], op=ALU.mult)
                    act(a1[:], sq1[:], AF.Exp, scale=-ALPHA)
                elif g % 2 == 1:
                    bt = bb_pool.tile([128, 1024], F32, tag="bb")
                    nc.vector.tensor_tensor(bt[:], b_prev[:], r1[:], op=ALU.mult)
                else:
                    bt = bb_pool.tile([128, 1024], F32, tag="bb")
                    act(bt[:], tt[:, sl], AF.Exp, scale=2.0 * ALPHA * c)
                b_prev = bt
                nc.vector.tensor_tensor(
                    e_h0[g][:, 1024:2048], a1[:], bt[:], op=ALU.mult)
            for g in range(G):
                mms(e_h0[g][:, 1024:2048], 8 + g, False)

            # --- h1 (ib2 + ib3): full-width basis, (g, ib) interleaved PE ---
            sl = slice(2048, 4096)
            sqh = sqh_pool.tile([128, 2048], F32, tag="sqh")
            ah = aah_pool.tile([128, 2048], F32, tag="aah")
            rh = rrh_pool.tile([128, 2048], F32, tag="rrh")
            act(tt[:, sl], xt_sb[:, sl], AF.Tanh)
            b_prev = None
            for g in range(G):
                c = float(CENTERS[g])
                e_t = ee_pool.tile([128, 2048], BF16, tag="ee")
                if g == 0:
                    bt = bbh_pool.tile([128, 2048], F32, tag="bbh")
                    act(bt[:], tt[:, sl], AF.Exp, scale=2.0 * ALPHA * c)
                    act(rh[:], tt[:, sl], AF.Exp, scale=R_SCALE)
                    nc.vector.tensor_tensor(
                        sqh[:], tt[:, sl], tt[:, sl], op=ALU.mult)
                    act(ah[:], sqh[:], AF.Exp, scale=-ALPHA)
                elif g % 2 == 1:
                    bt = bbh_pool.tile([128, 2048], F32, tag="bbh")
                    nc.vector.tensor_tensor(bt[:], b_prev[:], rh[:], op=ALU.mult)
                else:
                    bt = bbh_pool.tile([128, 2048], F32, tag="bbh")
                    act(bt[:], tt[:, sl], AF.Exp, scale=2.0 * ALPHA * c)
                b_prev = bt
                nc.vector.tensor_tensor(e_t[:], ah[:], bt[:], op=ALU.mult)
                for ib_l in range(2):
                    mms(e_t[:, ib_l * 1024:(ib_l + 1) * 1024],
                        16 + g * 2 + ib_l, False)

            # ---- drain psum -> SBUF bf16 (DVE bc0 / ACT bc1), 4 out DMAs ----
            o_sb = ob_pool.tile([128, N_OT * B_SH], BF16, tag="osb")
            for ot in range(N_OT):
                for bc in range(N_BC):
                    dst = o_sb[:, (ot * N_BC + bc) * 512:(ot * N_BC + bc + 1) * 512]
                    if bc == 0:
                        nc.vector.tensor_copy(dst, psum[ot][bc][:])
                    else:
                        di = nc.scalar.activation(dst, psum[ot][bc][:], AF.Copy)
                        add_dep_helper(di.ins, act_chain[-1].ins, sync=False,
                                       reason="ACT program order")
                        act_chain.append(di)
                out_eng = nc.sync if ot % 2 == 0 else nc.gpsimd
                out_eng.dma_start(
                    out_d[ot * 128:(ot + 1) * 128, :]
                    .rearrange("p (bc c) -> p bc c", c=512),
                    o_sb[:, ot * 1024:(ot + 1) * 1024]
                    .rearrange("p (bc c) -> p bc c", c=512),
                )
    nc.compile()
    return nc


def get_nc():
    if "nc" not in _NC_CACHE:
        _NC_CACHE["nc"] = build_nc()
    return _NC_CACHE["nc"]


def prep_inputs(x, weights, coefficients):
    x = np.asarray(x, dtype=np.float32)
    weights = np.asarray(weights, dtype=np.float32)
    coefficients = np.asarray(coefficients, dtype=np.float32)
    # W2T[k=g*I+i, o] = coeff[o,i,g] * W[o,i] * exp(-a c_g^2)
    w2t = (coefficients * weights[:, :, None]).transpose(2, 1, 0).reshape(K, O_SZ)
    gauss_bias = np.exp(-ALPHA * CENTERS ** 2)  # [G]
    w2t = (w2t.reshape(G, I_SZ, O_SZ) * gauss_bias[:, None, None]).astype(np.float32)
    # reorder k-tiles into device consumption order:
    #   h0: kt = ib*8 + g  (ib in 0,1)      <- ib-major
    #   h1: kt = 16 + g*2 + (ib-2)          <- g-major
    w2t = w2t.reshape(G, N_IBLK, 128, O_SZ)            # [g, ib, p, o]
    tiles = np.empty((32, 128, O_SZ), dtype=np.float32)
    for ib in range(2):
        for g in range(G):
            tiles[ib * 8 + g] = w2t[g, ib]
    for g in range(G):
        for ib in range(2):
            tiles[16 + g * 2 + ib] = w2t[g, 2 + ib]
    w2t_bf = tiles.reshape(K, O_SZ).astype(ml_dtypes.bfloat16)
    xT = np.ascontiguousarray(x.T)  # [I, B]
    in_maps = [
        {
            "xt": np.ascontiguousarray(xT[:, c * B_SH:(c + 1) * B_SH]),
            "w2t": w2t_bf,
        }
        for c in range(NCORES)
    ]
    return in_maps


def kernel(x, weights, coefficients):
    nc = get_nc()
    in_maps = prep_inputs(x, weights, coefficients)
    res = run_bass_kernel_spmd(nc, in_maps, core_ids=list(range(NCORES)))
    out = np.empty((B, O_SZ), dtype=np.float32)
    for c in range(NCORES):
        out[c * B_SH:(c + 1) * B_SH, :] = \
            res.results[c]["out_t"].astype(np.float32).T
    return out


# revision 10
# speedup vs baseline: 1.1663x; 1.1663x over previous
"""CustomGaussianLayer Trainium2 kernel (bf16, packed-DMA).

Math: out[b,o] = sum_{i,g} exp(-a*(tanh(x[b,i])-c_g)^2) * coeff[o,i,g]*W[o,i]
 == E @ W2T  with  E[b, k=(i,g)] Gaussian basis,  W2T[k, o] folded weights,
 a = 24.5, centers c_g = linspace(-1,1,8).

Factored basis:  exp(-a(t-c)^2) = A * B_g * exp(-a c^2)
  A   = exp(-a t^2)                 (ACT, bf16 out)
  B_g = exp(2 a c_g t)              even g: ACT exp (bf16); odd: B_{g-1}*r (DVE)
  r   = exp(2 a dc t) = exp(14 t)   (ACT, bf16),  exp(-a c^2) folded into W2T

Per core (data-parallel over batch, 1024 rows each): E/W2T bf16, PE fp32
into 8 psum banks [4 o-tiles x 2 b-chunks], out bf16 upcast on host.
Inputs are host-packed into the exact SBUF image so every DMA runs with
2-16KB contiguous descriptors on the Sync hardware queue.  PE warm-up
matmuls on memset scratch defeat the HAM cold clock.  First i-block runs
bc-major with 512-col basis slivers so real matmuls start ~10us.
"""

import numpy as np
import ml_dtypes

import concourse.bacc as bacc
import concourse.bass as bass
import concourse.mybir as mybir
import concourse.tile as tile
from concourse.bass_utils import run_bass_kernel_spmd
from concourse.tile import add_dep_helper

G = 8
I_SZ = 512
O_SZ = 512
B = 8192
NCORES = 8
B_SH = B // NCORES          # 1024 batch rows per core
K = I_SZ * G                # 4096 contraction
N_IBLK = I_SZ // 128        # 4 partition blocks of i
FREE = N_IBLK * B_SH        # 4096 free layout (i_blk, b)
N_OT = O_SZ // 128          # 4 output tiles
N_BC = B_SH // 512          # 2 batch chunks of 512 (psum free limit fp32)
N_KT = K // 128             # 32 k tiles

ALPHA = 24.5
N_WARMUP = 6                # dummy matmuls to beat the HAM cold clock
CENTERS = np.linspace(-1.0, 1.0, G).astype(np.float64)
DC = float(CENTERS[1] - CENTERS[0])          # 2/7
R_SCALE = float(2.0 * ALPHA * DC)            # 14.0

F32 = mybir.dt.float32
BF16 = mybir.dt.bfloat16
AF = mybir.ActivationFunctionType
ALU = mybir.AluOpType

_NC_CACHE = {}


def build_nc():
    nc = bacc.Bacc("TRN2", target_bir_lowering=False)
    # all tensors pre-packed on host into the SBUF image layout
    xt_d = nc.dram_tensor("xt", [128, FREE], F32, kind="ExternalInput")
    w2t_d = nc.dram_tensor("w2t", [128, N_KT * O_SZ], BF16, kind="ExternalInput")
    out_d = nc.dram_tensor("out_t", [128, N_OT * B_SH], BF16,
                           kind="ExternalOutput")

    with tile.TileContext(nc) as tc:
        with (
            tc.tile_pool(name="scr", bufs=1) as scr_pool,
            tc.tile_pool(name="w2", bufs=1) as w2_pool,
            tc.tile_pool(name="xt", bufs=1) as xt_pool,
            tc.tile_pool(name="tt", bufs=1) as tt_pool,
            tc.tile_pool(name="sq", bufs=2) as sq_pool,
            tc.tile_pool(name="rr", bufs=2) as rr_pool,
            tc.tile_pool(name="aa", bufs=2) as aa_pool,
            tc.tile_pool(name="bb", bufs=3) as bb_pool,
            tc.tile_pool(name="ee", bufs=8) as ee_pool,
            tc.tile_pool(name="ps", bufs=1, space="PSUM") as ps_pool,
            tc.tile_pool(name="ob", bufs=1) as ob_pool,
        ):
            # ---- warm-up scaffolding, all DMA-independent ----
            actwarm = scr_pool.tile([128, 2], F32, tag="actwarm")
            nc.vector.memset(actwarm[:, 0:1], 0.0)
            scr = scr_pool.tile([128, 640], BF16, tag="scr")
            nc.vector.memset(scr[:], 0.0)
            # ACT spline-table preload (exp/tanh share one table set)
            actwarm_i = nc.scalar.activation(
                actwarm[:, 1:2], actwarm[:, 0:1], AF.Exp)

            psum = [
                [
                    ps_pool.tile(
                        [128, 512], F32,
                        name=f"ps{ot}_{bc}", tag=f"ps{ot}_{bc}",
                    )
                    for bc in range(N_BC)
                ]
                for ot in range(N_OT)
            ]
            for w in range(N_WARMUP):
                nc.tensor.matmul(
                    psum[0][0][:], scr[:, 0:128], scr[:, 128:640],
                    start=(w == 0), stop=(w == N_WARMUP - 1),
                )

            # ---- input DMA: one Sync hw queue, consumer order ----
            w2_all = w2_pool.tile([128, N_KT * O_SZ], BF16, tag="w2all")
            xt_sb = xt_pool.tile([128, FREE], F32, tag="xt")

            def in_dma(dst, src, lo, hi):
                return nc.sync.dma_start(dst[:, lo:hi], src[lo:hi])

            dma_chain = [
                nc.sync.dma_start(xt_sb[:, 0:512], xt_d[:, 0:512]),
                nc.sync.dma_start(w2_all[:, 0:1024], w2t_d[:, 0:1024]),
                nc.sync.dma_start(xt_sb[:, 512:1024], xt_d[:, 512:1024]),
                nc.sync.dma_start(w2_all[:, 1024:4096], w2t_d[:, 1024:4096]),
                nc.sync.dma_start(xt_sb[:, 1024:2048], xt_d[:, 1024:2048]),
                nc.sync.dma_start(w2_all[:, 4096:8192], w2t_d[:, 4096:8192]),
                nc.sync.dma_start(xt_sb[:, 2048:4096], xt_d[:, 2048:4096]),
                nc.sync.dma_start(w2_all[:, 8192:16384], w2t_d[:, 8192:16384]),
            ]
            for i in range(1, len(dma_chain)):
                add_dep_helper(dma_chain[i].ins, dma_chain[i - 1].ins,
                               sync=False, reason="DMA lane consumer order")

            # ---- engine program-order chains ----
            act_chain = [actwarm_i]
            dve_chain = []

            def act(out_ap, in_ap, fn, scale=1.0, name=None):
                i = nc.scalar.activation(out_ap, in_ap, fn, scale=float(scale))
                add_dep_helper(i.ins, act_chain[-1].ins, sync=False,
                               reason="ACT program order")
                act_chain.append(i)
                return i

            def dve(out_ap, in0, in1):
                i = nc.vector.tensor_tensor(out_ap, in0, in1, op=ALU.mult)
                if dve_chain:
                    add_dep_helper(i.ins, dve_chain[-1].ins, sync=False,
                                   reason="DVE program order")
                dve_chain.append(i)
                return i

            # ---- basis helper: cols [lo,hi) of tt -> e_tiles[g][lo-off:hi-off]
            def basis(lo, hi, e_tiles, off, label):
                w = hi - lo
                sl = slice(lo, hi)
                sq_t = sq_pool.tile([128, w], F32, tag="sq", name=f"sq_{label}")
                a_t = aa_pool.tile([128, w], BF16, tag="aa", name=f"a_{label}")
                r_t = rr_pool.tile([128, w], BF16, tag="rr", name=f"r_{label}")
                b_prev = None
                act(tt[:, sl], xt_sb[:, sl], AF.Tanh)
                for g in range(G):
                    c = float(CENTERS[g])
                    if g % 2 == 0:
                        b_t = bb_pool.tile([128, w], BF16, tag="bb",
                                           name=f"b_{label}_{g}")
                        act(b_t[:], tt[:, sl], AF.Exp, scale=2.0 * ALPHA * c)
                        if g == 0:
                            dve(sq_t[:], tt[:, sl], tt[:, sl])
                            act(a_t[:], sq_t[:], AF.Exp, scale=-ALPHA)
                            act(r_t[:], tt[:, sl], AF.Exp, scale=R_SCALE)
                    else:
                        b_t = bb_pool.tile([128, w], BF16, tag="bb",
                                           name=f"b_{label}_{g}")
                        dve(b_t[:], b_prev[:], r_t[:])
                    b_prev = b_t
                    dve(e_tiles[g][:, lo - off:hi - off], a_t[:], b_t[:])

            # ---- matmul emission with per-bank start tracking ----
            started = set()
            mm_count = [0]
            TOTAL_MM = N_KT * N_OT * N_BC

            def mm4(kt, bc, e_ap512):
                """4 matmuls (all ot) into bank column bc."""
                for ot in range(N_OT):
                    lhsT = w2_all[:, kt * O_SZ + ot * 128:
                                  kt * O_SZ + (ot + 1) * 128]
                    first = (ot, bc) not in started
                    started.add((ot, bc))
                    last = mm_count[0] >= TOTAL_MM - N_OT * N_BC
                    nc.tensor.matmul(
                        psum[ot][bc][:], lhsT, e_ap512,
                        start=first, stop=last,
                    )
                    mm_count[0] += 1

            tt = tt_pool.tile([128, FREE], F32, tag="tt")
            e_h0 = [ee_pool.tile([128, 2048], BF16, tag="ee", name=f"e_h0_{g}")
                    for g in range(G)]

            # h0-ib0 basis in two 512 slivers, then ib1 full width
            basis(0, 512, e_h0, 0, "s0")
            # PE pass 1: ib0 bc0 (sliver a)
            for g in range(G):
                mm4(g, 0, e_h0[g][:, 0:512])
            basis(512, 1024, e_h0, 0, "s1")
            # PE pass 2: ib0 bc1 (sliver b)
            for g in range(G):
                mm4(g, 1, e_h0[g][:, 512:1024])
            basis(1024, 2048, e_h0, 0, "s2")
            # PE pass 3: ib1 both bc
            for g in range(G):
                for bc in range(N_BC):
                    mm4(8 + g, bc, e_h0[g][:, 1024 + bc * 512:1536 + bc * 512])

            # h1: full-width basis per g, (g, ib) interleaved PE
            sqh = sq_pool.tile([128, 2048], F32, tag="sqh")
            ah = aa_pool.tile([128, 2048], BF16, tag="aah")
            rh = rr_pool.tile([128, 2048], BF16, tag="rrh")
            sl = slice(2048, 4096)
            act(tt[:, sl], xt_sb[:, sl], AF.Tanh)
            b_prev = None
            for g in range(G):
                c = float(CENTERS[g])
                e_t = ee_pool.tile([128, 2048], BF16, tag="ee",
                                   name=f"e_h1_{g}")
                if g % 2 == 0:
                    b_t = bb_pool.tile([128, 2048], BF16, tag="bbh",
                                       name=f"bh_{g}")
                    act(b_t[:], tt[:, sl], AF.Exp, scale=2.0 * ALPHA * c)
                    if g == 0:
                        dve(sqh[:], tt[:, sl], tt[:, sl])
                        act(ah[:], sqh[:], AF.Exp, scale=-ALPHA)
                        act(rh[:], tt[:, sl], AF.Exp, scale=R_SCALE)
                else:
                    b_t = bb_pool.tile([128, 2048], BF16, tag="bbh",
                                       name=f"bh_{g}")
                    dve(b_t[:], b_prev[:], rh[:])
                b_prev = b_t
                dve(e_t[:], ah[:], b_t[:])
                for ib_l in range(2):
                    for bc in range(N_BC):
                        mm4(16 + g * 2 + ib_l, bc,
                            e_t[:, ib_l * 1024 + bc * 512:
                                ib_l * 1024 + bc * 512 + 512])

            # ---- drain psum -> SBUF bf16 (DVE bc0 / ACT bc1), out DMAs ----
            o_sb = ob_pool.tile([128, N_OT * B_SH], BF16, tag="osb")
            for ot in range(N_OT):
                for bc in range(N_BC):
                    dst = o_sb[:, (ot * N_BC + bc) * 512:
                               (ot * N_BC + bc + 1) * 512]
                    if bc == 0:
                        ci = nc.vector.tensor_copy(dst, psum[ot][bc][:])
                        add_dep_helper(ci.ins, dve_chain[-1].ins, sync=False,
                                       reason="DVE program order")
                        dve_chain.append(ci)
                    else:
                        di = nc.scalar.activation(dst, psum[ot][bc][:], AF.Copy)
                        add_dep_helper(di.ins, act_chain[-1].ins, sync=False,
                                       reason="ACT program order")
                        act_chain.append(di)
                out_eng = nc.sync if ot % 2 == 0 else nc.scalar
                out_eng.dma_start(
                    out_d[:, ot * B_SH:(ot + 1) * B_SH],
                    o_sb[:, ot * B_SH:(ot + 1) * B_SH],
                )
    nc.compile()
    return nc


def get_nc():
    if "nc" not in _NC_CACHE:
        _NC_CACHE["nc"] = build_nc()
    return _NC_CACHE["nc"]


def prep_inputs(x, weights, coefficients):
    x = np.asarray(x, dtype=np.float32)
    weights = np.asarray(weights, dtype=np.float32)
    coefficients = np.asarray(coefficients, dtype=np.float32)
    # W2T[k=g*I+i, o] = coeff[o,i,g] * W[o,i] * exp(-a c_g^2)
    w2t = (coefficients * weights[:, :, None]).transpose(2, 1, 0).reshape(K, O_SZ)
    gauss_bias = np.exp(-ALPHA * CENTERS ** 2)  # [G]
    w2t = (w2t.reshape(G, I_SZ, O_SZ) * gauss_bias[:, None, None]).astype(np.float32)
    # k-tile order: h0: kt = ib*8 + g (ib 0,1); h1: kt = 16 + g*2 + (ib-2)
    w2t = w2t.reshape(G, N_IBLK, 128, O_SZ)            # [g, ib, p, o]
    tiles = np.empty((N_KT, 128, O_SZ), dtype=np.float32)
    for ib in range(2):
        for g in range(G):
            tiles[ib * 8 + g] = w2t[g, ib]
    for g in range(G):
        for ib in range(2):
            tiles[16 + g * 2 + ib] = w2t[g, 2 + ib]
    # pack to SBUF image [128p, kt*O]
    w2_img = np.ascontiguousarray(
        tiles.transpose(1, 0, 2).reshape(128, N_KT * O_SZ)
    ).astype(ml_dtypes.bfloat16)
    xT = np.ascontiguousarray(x.T)  # [I, B]
    in_maps = []
    for c in range(NCORES):
        xc = xT[:, c * B_SH:(c + 1) * B_SH]            # [512, 1024]
        xt_img = np.ascontiguousarray(
            xc.reshape(N_IBLK, 128, B_SH).transpose(1, 0, 2).reshape(128, FREE)
        )
        in_maps.append({"xt": xt_img, "w2t": w2_img})
    return in_maps


def unpack_out(res):
    out = np.empty((B, O_SZ), dtype=np.float32)
    for c in range(NCORES):
        o_img = np.asarray(res.results[c]["out_t"]).astype(np.float32)
        # [128, ot*1024+b] -> [O, B_SH] -> [B_SH, O]
        o_full = o_img.reshape(128, N_OT, B_SH).transpose(1, 0, 2) \
                      .reshape(O_SZ, B_SH)
        out[c * B_SH:(c + 1) * B_SH, :] = o_full.T
    return out


def kernel(x, weights, coefficients):
    nc = get_nc()
    in_maps = prep_inputs(x, weights, coefficients)
    res = run_bass_kernel_spmd(nc, in_maps, core_ids=list(range(NCORES)))
    return unpack_out(res)


# revision 15
# speedup vs baseline: 1.1890x; 1.0195x over previous
"""CustomGaussianLayer Trainium2 kernel (bf16, packed-DMA).

Math: out[b,o] = sum_{i,g} exp(-a*(tanh(x[b,i])-c_g)^2) * coeff[o,i,g]*W[o,i]
 == E @ W2T  with  E[b, k=(i,g)] Gaussian basis,  W2T[k, o] folded weights,
 a = 24.5, centers c_g = linspace(-1,1,8).

Factored basis:  exp(-a(t-c)^2) = A * B_g * exp(-a c^2)
  A   = exp(-a t^2)                 (ACT, bf16 out)
  B_g = exp(2 a c_g t)              even g: ACT exp (bf16); odd: B_{g-1}*r (DVE)
  r   = exp(2 a dc t) = exp(14 t)   (ACT, bf16),  exp(-a c^2) folded into W2T

Per core (data-parallel over batch, 1024 rows each): E/W2T bf16, PE fp32
into 8 psum banks [4 o-tiles x 2 b-chunks], out bf16 upcast on host.
Inputs are host-packed into the exact SBUF image so every DMA runs with
2-16KB contiguous descriptors on the Sync hardware queue.  PE warm-up
matmuls on memset scratch defeat the HAM cold clock.  First i-block runs
bc-major with 512-col basis slivers so real matmuls start ~10us.
"""

import numpy as np
import ml_dtypes

import concourse.bacc as bacc
import concourse.bass as bass
import concourse.mybir as mybir
import concourse.tile as tile
from concourse.bass_utils import run_bass_kernel_spmd
from concourse.tile import add_dep_helper

G = 8
I_SZ = 512
O_SZ = 512
B = 8192
NCORES = 8
B_SH = B // NCORES          # 1024 batch rows per core
K = I_SZ * G                # 4096 contraction
N_IBLK = I_SZ // 128        # 4 partition blocks of i
FREE = N_IBLK * B_SH        # 4096 free layout (i_blk, b)
N_OT = O_SZ // 128          # 4 output tiles
N_BC = B_SH // 512          # 2 batch chunks of 512 (psum free limit fp32)
N_KT = K // 128             # 32 k tiles

ALPHA = 24.5
N_WARMUP = 11               # dummy matmuls to beat the HAM cold clock
CENTERS = np.linspace(-1.0, 1.0, G).astype(np.float64)
DC = float(CENTERS[1] - CENTERS[0])          # 2/7
R_SCALE = float(2.0 * ALPHA * DC)            # 14.0

F32 = mybir.dt.float32
BF16 = mybir.dt.bfloat16
AF = mybir.ActivationFunctionType
ALU = mybir.AluOpType

_NC_CACHE = {}


def build_nc():
    nc = bacc.Bacc("TRN2", target_bir_lowering=False)
    # all tensors pre-packed on host into the SBUF image layout
    xt_d = nc.dram_tensor("xt", [128, FREE], F32, kind="ExternalInput")
    w2t_d = nc.dram_tensor("w2t", [128, N_KT * O_SZ], BF16, kind="ExternalInput")
    out_d = nc.dram_tensor("out_t", [128, N_OT * B_SH], BF16,
                           kind="ExternalOutput")

    with tile.TileContext(nc) as tc:
        with (
            tc.tile_pool(name="scr", bufs=1) as scr_pool,
            tc.tile_pool(name="w2", bufs=1) as w2_pool,
            tc.tile_pool(name="xt", bufs=1) as xt_pool,
            tc.tile_pool(name="tt", bufs=1) as tt_pool,
            tc.tile_pool(name="sq", bufs=2) as sq_pool,
            tc.tile_pool(name="rr", bufs=2) as rr_pool,
            tc.tile_pool(name="aa", bufs=2) as aa_pool,
            tc.tile_pool(name="bb", bufs=3) as bb_pool,
            tc.tile_pool(name="ee", bufs=8) as ee_pool,
            tc.tile_pool(name="ps", bufs=1, space="PSUM") as ps_pool,
            tc.tile_pool(name="ob", bufs=1) as ob_pool,
        ):
            # ---- warm-up scaffolding, all DMA-independent ----
            w2_all = w2_pool.tile([128, N_KT * O_SZ], BF16, tag="w2all")
            xt_sb = xt_pool.tile([128, FREE], F32, tag="xt")
            # first xt sliver rides the Scalar hw queue: issued before the
            # ACT table load, in parallel with Sync's input lane
            xt_a_dma = nc.scalar.dma_start(xt_sb[:, 0:512], xt_d[:, 0:512])

            actwarm = scr_pool.tile([128, 2], F32, tag="actwarm")
            nc.vector.memset(actwarm[:, 0:1], 0.0)
            scr = scr_pool.tile([128, 640], BF16, tag="scr")
            nc.vector.memset(scr[:], 0.0)
            # ACT spline-table preload (exp/tanh share one table set)
            actwarm_i = nc.scalar.activation(
                actwarm[:, 1:2], actwarm[:, 0:1], AF.Exp)
            add_dep_helper(actwarm_i.ins, xt_a_dma.ins, sync=False,
                           reason="scalar queue: xt dma before table load")

            psum = [
                [
                    ps_pool.tile(
                        [128, 512], F32,
                        name=f"ps{ot}_{bc}", tag=f"ps{ot}_{bc}",
                    )
                    for bc in range(N_BC)
                ]
                for ot in range(N_OT)
            ]
            for w in range(N_WARMUP):
                nc.tensor.matmul(
                    psum[0][0][:], scr[:, 0:128], scr[:, 128:640],
                    start=(w == 0), stop=(w == N_WARMUP - 1),
                )

            # ---- input DMA: Sync hw queue, consumer order ----
            dma_chain = [
                nc.sync.dma_start(w2_all[:, 0:1024], w2t_d[:, 0:1024]),
                nc.sync.dma_start(xt_sb[:, 512:1024], xt_d[:, 512:1024]),
                nc.sync.dma_start(xt_sb[:, 1024:2048], xt_d[:, 1024:2048]),
                nc.sync.dma_start(w2_all[:, 1024:4096], w2t_d[:, 1024:4096]),
                nc.sync.dma_start(xt_sb[:, 2048:4096], xt_d[:, 2048:4096]),
                nc.sync.dma_start(w2_all[:, 4096:8192], w2t_d[:, 4096:8192]),
                nc.sync.dma_start(w2_all[:, 8192:16384], w2t_d[:, 8192:16384]),
            ]
            for i in range(1, len(dma_chain)):
                add_dep_helper(dma_chain[i].ins, dma_chain[i - 1].ins,
                               sync=False, reason="DMA lane consumer order")

            # ---- engine program-order chains ----
            act_chain = [actwarm_i]
            dve_chain = []

            def act(out_ap, in_ap, fn, scale=1.0, name=None):
                i = nc.scalar.activation(out_ap, in_ap, fn, scale=float(scale))
                add_dep_helper(i.ins, act_chain[-1].ins, sync=False,
                               reason="ACT program order")
                act_chain.append(i)
                return i

            def dve(out_ap, in0, in1):
                i = nc.vector.tensor_tensor(out_ap, in0, in1, op=ALU.mult)
                if dve_chain:
                    add_dep_helper(i.ins, dve_chain[-1].ins, sync=False,
                                   reason="DVE program order")
                dve_chain.append(i)
                return i

            # ---- basis helper: cols [lo,hi) of tt -> e_tiles[g][lo-off:hi-off]
            def basis(lo, hi, e_tiles, off, label):
                w = hi - lo
                sl = slice(lo, hi)
                sq_t = sq_pool.tile([128, w], F32, tag="sq", name=f"sq_{label}")
                a_t = aa_pool.tile([128, w], BF16, tag="aa", name=f"a_{label}")
                r_t = rr_pool.tile([128, w], BF16, tag="rr", name=f"r_{label}")
                b_prev = None
                act(tt[:, sl], xt_sb[:, sl], AF.Tanh)
                for g in range(G):
                    c = float(CENTERS[g])
                    if g % 2 == 0:
                        b_t = bb_pool.tile([128, w], BF16, tag="bb",
                                           name=f"b_{label}_{g}")
                        act(b_t[:], tt[:, sl], AF.Exp, scale=2.0 * ALPHA * c)
                        if g == 0:
                            dve(sq_t[:], tt[:, sl], tt[:, sl])
                            act(a_t[:], sq_t[:], AF.Exp, scale=-ALPHA)
                            act(r_t[:], tt[:, sl], AF.Exp, scale=R_SCALE)
                    else:
                        b_t = bb_pool.tile([128, w], BF16, tag="bb",
                                           name=f"b_{label}_{g}")
                        dve(b_t[:], b_prev[:], r_t[:])
                    b_prev = b_t
                    dve(e_tiles[g][:, lo - off:hi - off], a_t[:], b_t[:])

            # ---- matmul emission with per-bank start tracking ----
            started = set()
            mm_count = [0]
            TOTAL_MM = N_KT * N_OT * N_BC

            def mm4(kt, bc, e_ap512):
                """4 matmuls (all ot) into bank column bc."""
                for ot in range(N_OT):
                    lhsT = w2_all[:, kt * O_SZ + ot * 128:
                                  kt * O_SZ + (ot + 1) * 128]
                    first = (ot, bc) not in started
                    started.add((ot, bc))
                    last = mm_count[0] >= TOTAL_MM - N_OT * N_BC
                    nc.tensor.matmul(
                        psum[ot][bc][:], lhsT, e_ap512,
                        start=first, stop=last,
                    )
                    mm_count[0] += 1

            tt = tt_pool.tile([128, FREE], F32, tag="tt")
            e_h0 = [ee_pool.tile([128, 2048], BF16, tag="ee", name=f"e_h0_{g}")
                    for g in range(G)]

            # h0-ib0 basis in two 512 slivers, then ib1 full width
            basis(0, 512, e_h0, 0, "s0")
            # PE pass 1: ib0 bc0 (sliver a)
            for g in range(G):
                mm4(g, 0, e_h0[g][:, 0:512])
            basis(512, 1024, e_h0, 0, "s1")
            # PE pass 2: ib0 bc1 (sliver b)
            for g in range(G):
                mm4(g, 1, e_h0[g][:, 512:1024])
            basis(1024, 2048, e_h0, 0, "s2")
            # PE pass 3: ib1 both bc
            for g in range(G):
                for bc in range(N_BC):
                    mm4(8 + g, bc, e_h0[g][:, 1024 + bc * 512:1536 + bc * 512])

            # h1: full-width basis per g, (g, ib) interleaved PE
            sqh = sq_pool.tile([128, 2048], F32, tag="sqh")
            ah = aa_pool.tile([128, 2048], BF16, tag="aah")
            rh = rr_pool.tile([128, 2048], BF16, tag="rrh")
            sl = slice(2048, 4096)
            act(tt[:, sl], xt_sb[:, sl], AF.Tanh)
            b_prev = None
            e_h1 = {}
            for g in range(G):
                c = float(CENTERS[g])
                e_t = ee_pool.tile([128, 2048], BF16, tag="ee",
                                   name=f"e_h1_{g}")
                e_h1[g] = e_t
                if g % 2 == 0:
                    b_t = bb_pool.tile([128, 2048], BF16, tag="bbh",
                                       name=f"bh_{g}")
                    act(b_t[:], tt[:, sl], AF.Exp, scale=2.0 * ALPHA * c)
                    if g == 0:
                        dve(sqh[:], tt[:, sl], tt[:, sl])
                        act(ah[:], sqh[:], AF.Exp, scale=-ALPHA)
                        act(rh[:], tt[:, sl], AF.Exp, scale=R_SCALE)
                else:
                    b_t = bb_pool.tile([128, 2048], BF16, tag="bbh",
                                       name=f"bh_{g}")
                    dve(b_t[:], b_prev[:], rh[:])
                b_prev = b_t
                dve(e_t[:], ah[:], b_t[:])
                if g < G - 2:
                    for ib_l in range(2):
                        for bc in range(N_BC):
                            mm4(16 + g * 2 + ib_l, bc,
                                e_t[:, ib_l * 1024 + bc * 512:
                                    ib_l * 1024 + bc * 512 + 512])

            # last two g-blocks bank-major: each bank retires early so its
            # drain + output DMA overlap the remaining matmuls
            o_sb = ob_pool.tile([128, N_OT * B_SH], BF16, tag="osb")
            for ot in range(N_OT):
                for bc in range(N_BC):
                    for g in (G - 2, G - 1):
                        for ib_l in range(2):
                            kt = 16 + g * 2 + ib_l
                            lhsT = w2_all[:, kt * O_SZ + ot * 128:
                                          kt * O_SZ + (ot + 1) * 128]
                            nc.tensor.matmul(
                                psum[ot][bc][:], lhsT,
                                e_h1[g][:, ib_l * 1024 + bc * 512:
                                        ib_l * 1024 + bc * 512 + 512],
                                start=False,
                                stop=(g == G - 1 and ib_l == 1),
                            )
                    dst = o_sb[:, (ot * N_BC + bc) * 512:
                               (ot * N_BC + bc + 1) * 512]
                    if bc == 0:
                        ci = nc.vector.tensor_copy(dst, psum[ot][bc][:])
                        add_dep_helper(ci.ins, dve_chain[-1].ins, sync=False,
                                       reason="DVE program order")
                        dve_chain.append(ci)
                    else:
                        di = nc.scalar.activation(dst, psum[ot][bc][:], AF.Copy)
                        add_dep_helper(di.ins, act_chain[-1].ins, sync=False,
                                       reason="ACT program order")
                        act_chain.append(di)
                out_eng = nc.sync if ot % 2 == 0 else nc.scalar
                out_eng.dma_start(
                    out_d[:, ot * B_SH:(ot + 1) * B_SH],
                    o_sb[:, ot * B_SH:(ot + 1) * B_SH],
                )
    nc.compile()
    return nc


def get_nc():
    if "nc" not in _NC_CACHE:
        _NC_CACHE["nc"] = build_nc()
    return _NC_CACHE["nc"]


def prep_inputs(x, weights, coefficients):
    x = np.asarray(x, dtype=np.float32)
    weights = np.asarray(weights, dtype=np.float32)
    coefficients = np.asarray(coefficients, dtype=np.float32)
    # W2T[k=g*I+i, o] = coeff[o,i,g] * W[o,i] * exp(-a c_g^2)
    w2t = (coefficients * weights[:, :, None]).transpose(2, 1, 0).reshape(K, O_SZ)
    gauss_bias = np.exp(-ALPHA * CENTERS ** 2)  # [G]
    w2t = (w2t.reshape(G, I_SZ, O_SZ) * gauss_bias[:, None, None]).astype(np.float32)
    # k-tile order: h0: kt = ib*8 + g (ib 0,1); h1: kt = 16 + g*2 + (ib-2)
    w2t = w2t.reshape(G, N_IBLK, 128, O_SZ)            # [g, ib, p, o]
    tiles = np.empty((N_KT, 128, O_SZ), dtype=np.float32)
    for ib in range(2):
        for g in range(G):
            tiles[ib * 8 + g] = w2t[g, ib]
    for g in range(G):
        for ib in range(2):
            tiles[16 + g * 2 + ib] = w2t[g, 2 + ib]
    # pack to SBUF image [128p, kt*O]
    w2_img = np.ascontiguousarray(
        tiles.transpose(1, 0, 2).reshape(128, N_KT * O_SZ)
    ).astype(ml_dtypes.bfloat16)
    xT = np.ascontiguousarray(x.T)  # [I, B]
    in_maps = []
    for c in range(NCORES):
        xc = xT[:, c * B_SH:(c + 1) * B_SH]            # [512, 1024]
        xt_img = np.ascontiguousarray(
            xc.reshape(N_IBLK, 128, B_SH).transpose(1, 0, 2).reshape(128, FREE)
        )
        in_maps.append({"xt": xt_img, "w2t": w2_img})
    return in_maps


def unpack_out(res):
    out = np.empty((B, O_SZ), dtype=np.float32)
    for c in range(NCORES):
        o_img = np.asarray(res.results[c]["out_t"]).astype(np.float32)
        # [128, ot*1024+b] -> [O, B_SH] -> [B_SH, O]
        o_full = o_img.reshape(128, N_OT, B_SH).transpose(1, 0, 2) \
                      .reshape(O_SZ, B_SH)
        out[c * B_SH:(c + 1) * B_SH, :] = o_full.T
    return out


def kernel(x, weights, coefficients):
    nc = get_nc()
    in_maps = prep_inputs(x, weights, coefficients)
    res = run_bass_kernel_spmd(nc, in_maps, core_ids=list(range(NCORES)))
    return unpack_out(res)
